# revision 1
# baseline (speedup 1.0000x reference)
"""Head-sharded causal self-attention (QK-RMSNorm + RoPE + value-residual mix)
for 8 Trainium2 NeuronCores.

Sharding: 16 heads -> 2 heads per core (tensor parallel). Each core computes
its heads' QKV projections, attention, and a partial c_proj output
[D, T] (transposed); the host sums the 8 partials (the c_proj all-reduce).

Layout strategy (per core):
 - QKV matmul produces q,k,v in natural [T, hd] tiles (lhsT = x^T tiles,
   moving = W_all^T), so RMS-norm + RoPE run with T on partitions.
 - q,k are PE-transposed to [hd, T] for the score matmuls.
 - Scores are computed transposed: S^T[T_k, T_q] = k^T_tile.T @ q^T. The
   exp(S^T) tiles then directly serve as the moving operand of both the
   softmax-denominator ones-matmul and the o^T = v.T @ expS accumulation,
   so no attention-weight transpose is ever needed.
 - Causal mask: a DVE multiply zeroes masked entries of the exp tiles on
   the four diagonal-band k-tiles (0/1 band mask, sliced per offset).
 - k's RMS-norm scale rides for free as the per-partition `scale` operand
   of the exp activation; q's scale (together with 1/sqrt(hd)) is applied
   to q before RoPE.
 - o^T[hd, T_q] feeds c_proj directly: partial^T = Wproj_c^T.T @ o^T.
All matmuls run in float32r (1 cycle/row at N>=256, ~TF32 accuracy).
"""

import numpy as np

import concourse.bacc as bacc
import concourse.mybir as mybir
import concourse.tile as tile
from concourse.bass_utils import run_bass_kernel_spmd

P = 128
T = 4096
D = 2048
HD = 128
NH = 16
HPC = 2            # heads per core
NCORES = 8
NT = T // P        # 32 t-tiles
KT = D // P        # 16 contraction tiles for the projections
NJ = 8             # q-blocks
QB = 512           # q-block width
EPS = 1.1920929e-07
NEG = -1.0e30

F32 = mybir.dt.float32
F32R = mybir.dt.float32r


def _build():
    nc = bacc.Bacc("TRN2", target_bir_lowering=False, debug=False,
                   enable_asserts=False, num_devices=NCORES)

    # ---- DRAM parameters (host pre-tiled layouts) ----
    xt = nc.dram_tensor("xt", [NT, P, KT, P], F32R, kind="ExternalInput").ap()
    wall = nc.dram_tensor("wall", [P, KT, 6 * HD], F32R, kind="ExternalInput").ap()
    wproj = nc.dram_tensor("wproj", [P, HPC, D], F32R, kind="ExternalInput").ap()
    vilam = nc.dram_tensor("vilam", [NT, P, HPC * HD], F32, kind="ExternalInput").ap()
    cs = nc.dram_tensor("cs", [P, NT, HD], F32, kind="ExternalInput").ap()
    mask01 = nc.dram_tensor("mask01", [P, 896], F32R, kind="ExternalInput").ap()
    ones = nc.dram_tensor("ones", [P, 1], F32R, kind="ExternalInput").ap()
    identr = nc.dram_tensor("identr", [P, P], F32R, kind="ExternalInput").ap()
    out = nc.dram_tensor("out", [D, T], F32, kind="ExternalOutput").ap()

    with tile.TileContext(nc) as tc:
        with tc.tile_pool(name="persist", bufs=1) as persist:
            # tensors that live for the whole kernel
            qT = persist.tile([P, HPC, T], F32R)        # q^T per head
            kT_ = persist.tile([P, HPC, T], F32R)       # k^T per head
            v_sb = persist.tile([P, HPC, NT, HD], F32R)  # v natural per head
            cs_sbc = [persist.tile([P, 8, HD], F32, name=f"cs_sb{c}")
                      for c in range(4)]
            ident = persist.tile([P, P], F32R)
            eps_q = persist.tile([P, 1], F32)
            eps_k = persist.tile([P, 1], F32)
            nc.sync.dma_start(out=ident[:], in_=identr[:])
            nc.gpsimd.memset(eps_q[:], float(P) * EPS)
            nc.gpsimd.memset(eps_k[:], EPS)
            # warm up the gpsimd partition_broadcast ucode path early: its
            # first invocation pays a ~7us pool reconfig we must keep off
            # the phase-2 critical path
            warm = persist.tile([P, 8], F32)
            nc.gpsimd.partition_broadcast(warm[:], eps_q[0:1, 0:1]
                                          .broadcast_to([1, 8]))

            # ---------------- Phase 1: QKV + norm + rope + transposes ----
            with tc.tile_pool(name="p1w", bufs=1) as p1w, \
                 tc.tile_pool(name="p1sb", bufs=3) as p1sb, \
                 tc.tile_pool(name="p1sc", bufs=3) as p1sc, \
                 tc.tile_pool(name="p1ps", bufs=4, space="PSUM") as p1ps, \
                 tc.tile_pool(name="p1tp", bufs=3, space="PSUM") as p1tp:
                # per-kt weight tiles: tile-granular deps let the first
                # matmuls start as soon as their own chunk has landed
                wall_sb = [p1w.tile([P, 6 * HD], F32R, name=f"wall_sb{kt}")
                           for kt in range(KT)]
                nc.sync.dma_start(out=wall_sb[0][:], in_=wall[:, 0, :])
                # first x tile in 4 chunks so the very first matmuls
                # (tile-granular deps) start after ~1/4 of the transfer
                x_t0c = [p1w.tile([P, 4, P], F32R, name=f"x_t0c{c}")
                         for c in range(4)]
                for c in range(4):
                    nc.sync.dma_start(out=x_t0c[c][:],
                                      in_=xt[0, :, 4 * c:4 * (c + 1), :])
                    if c < 3:
                        nc.sync.dma_start(out=wall_sb[c + 1][:],
                                          in_=wall[:, c + 1, :])
                vi0 = p1sb.tile([P, HPC * HD], F32, tag="vi", name="vi0")
                nc.sync.dma_start(out=vi0[:], in_=vilam[0])
                for kt in range(4, KT):
                    nc.sync.dma_start(out=wall_sb[kt][:], in_=wall[:, kt, :])
                x1 = p1sb.tile([P, KT, P], F32R, tag="x", bufs=2, name="x1")
                nc.sync.dma_start(out=x1[:], in_=xt[1])
                nc.sync.dma_start(out=cs_sbc[0][:], in_=cs[:, 0:8, :])

                for tt in range(NT):
                    if 1 <= tt <= 3:
                        nc.sync.dma_start(out=cs_sbc[tt][:],
                                          in_=cs[:, 8 * tt:8 * (tt + 1), :])
                    if tt == 0:
                        def xop(kt):
                            return x_t0c[kt // 4][:, kt % 4, :]
                    elif tt == 1:
                        def xop(kt, x_t=x1):
                            return x_t[:, kt, :]
                    else:
                        x_t = p1sb.tile([P, KT, P], F32R, tag="x", bufs=2)
                        nc.sync.dma_start(out=x_t[:], in_=xt[tt])

                        def xop(kt, x_t=x_t):
                            return x_t[:, kt, :]
                    if tt == 0:
                        vi_t = vi0
                    else:
                        vi_t = p1sb.tile([P, HPC * HD], F32, tag="vi")
                        nc.sync.dma_start(out=vi_t[:], in_=vilam[tt])

                    halves = []
                    for half in range(2):
                        ps = p1ps.tile([P, 384], F32, tag="qkvps")
                        for kt in range(KT):
                            nc.tensor.matmul(
                                ps[:],
                                xop(kt),
                                wall_sb[kt][:, half * 384:(half + 1) * 384],
                                start=(kt == 0),
                                stop=(kt == KT - 1),
                            )
                        halves.append(ps)

                    # --- evict q,k into natural tile + per-row sum-squares
                    qk_nat = p1sb.tile([P, 4, P], F32, tag="qknat")
                    ssq = p1sc.tile([P, 4], F32, tag="ssq")
                    sqs = p1sb.tile([P, P], F32, tag="sqscratch")
                    nc.scalar.copy(qk_nat[:, 0:3, :], halves[0][:, 0:384])
                    nc.scalar.copy(qk_nat[:, 3, :], halves[1][:, 0:P])
                    for i in range(4):          # q0 q1 k0 k1
                        src = halves[i // 3][:, (i % 3) * P:(i % 3 + 1) * P]
                        nc.scalar.activation(
                            sqs[:], src, mybir.ActivationFunctionType.Square,
                            accum_out=ssq[:, i:i + 1])
                    # --- v: psum + lam*vi -> f32r natural tile
                    for h in range(HPC):
                        nc.vector.tensor_add(
                            v_sb[:, h, tt, :],
                            halves[1][:, P + h * P:P + (h + 1) * P],
                            vi_t[:, h * P:(h + 1) * P])

                    # --- rms scales: q -> 1/sqrt(ssq+128eps) (incl 1/sqrt(hd));
                    #     k -> 1/sqrt(ssq/128+eps)
                    # one sqrt for all four: 1/sqrt(ssq+128eps); k rows also
                    # need a sqrt(128) factor (k-norm lacks the 1/sqrt(hd))
                    sca = p1sc.tile([P, 4], F32, tag="sca")
                    rsc = p1sc.tile([P, 4], F32, tag="rsc")
                    nc.scalar.activation(sca[:], ssq[:],
                                         mybir.ActivationFunctionType.Sqrt,
                                         bias=eps_q[:], scale=1.0)
                    nc.vector.reciprocal(rsc[:], sca[:])

                    # --- scale q and k by their rms scales
                    for i in range(4):
                        nc.vector.tensor_scalar(
                            out=qk_nat[:, i, :], in0=qk_nat[:, i, :],
                            scalar1=rsc[:, i:i + 1],
                            scalar2=(1.0 if i < 2 else float(np.sqrt(P))),
                            op0=mybir.AluOpType.mult,
                            op1=mybir.AluOpType.mult)

                    # --- rope on all 4 tensors at once (f32r out: final
                    # values ahead of the f32r transpose + score matmuls)
                    rp = p1sb.tile([P, 4, P], F32R, tag="rope")
                    tmp = p1sb.tile([P, 4, 64], F32, tag="ropetmp")
                    x1_ = qk_nat[:, :, 0:64]
                    x2 = qk_nat[:, :, 64:128]
                    cst = cs_sbc[tt // 8]
                    cb = cst[:, tt % 8, None, 0:64].broadcast_to([P, 4, 64])
                    sb = cst[:, tt % 8, None, 64:128].broadcast_to([P, 4, 64])
                    nc.vector.tensor_mul(rp[:, :, 0:64], x1_, cb)
                    nc.vector.tensor_mul(tmp[:], x2, sb)
                    nc.vector.tensor_add(rp[:, :, 0:64], rp[:, :, 0:64], tmp[:])
                    nc.vector.tensor_mul(rp[:, :, 64:128], x2, cb)
                    nc.vector.tensor_mul(tmp[:], x1_, sb)
                    nc.vector.tensor_sub(rp[:, :, 64:128], rp[:, :, 64:128], tmp[:])

                    # --- transpose q,k tiles to [hd, T] layout
                    for i in range(4):
                        tp = p1tp.tile([P, P], F32R, tag="tp")
                        nc.tensor.transpose(tp[:], rp[:, i, :], ident[:])
                        dst = qT if i < 2 else kT_
                        nc.scalar.copy(dst[:, i % 2, tt * P:(tt + 1) * P], tp[:])

            # ---------------- Phase 2+3: attention + c_proj ----
            with tc.tile_pool(name="p2c", bufs=1) as p2c, \
                 tc.tile_pool(name="p2exp", bufs=6) as p2exp, \
                 tc.tile_pool(name="p2sb", bufs=4) as p2sb, \
                 tc.tile_pool(name="p2sc", bufs=4) as p2sc, \
                 tc.tile_pool(name="sps", bufs=4, space="PSUM") as sps, \
                 tc.tile_pool(name="ops", bufs=2, space="PSUM") as ops_, \
                 tc.tile_pool(name="sums", bufs=2, space="PSUM") as sums:
                mask_sb = p2c.tile([P, 896], F32R)
                ones_sb = p2c.tile([P, 1], F32R)
                wproj_sb = p2c.tile([P, HPC, D], F32R)
                nc.sync.dma_start(out=mask_sb[:], in_=mask01[:])
                nc.sync.dma_start(out=ones_sb[:], in_=ones[:])
                nc.sync.dma_start(out=wproj_sb[:], in_=wproj[:])

                def emit_proj(j, o_sb_pair):
                    for dt_ in range(KT):
                        pp = sps.tile([P, QB], F32, tag="sps",
                                      name=f"pp_{j}_{dt_}")
                        for h in range(HPC):
                            nc.tensor.matmul(
                                pp[:],
                                wproj_sb[:, h, dt_ * P:(dt_ + 1) * P],
                                o_sb_pair[h][:],
                                start=(h == 0), stop=(h == HPC - 1))
                        po = p2sb.tile([P, QB], F32, tag="po",
                                       name=f"po_{j}_{dt_}")
                        # alternate eviction engine so neither ACT (exp) nor
                        # DVE (softmax-normalize chain) gets head-of-line
                        # blocked behind a burst of proj evictions
                        if dt_ % 2 == 0:
                            nc.vector.tensor_copy(po[:], pp[:])
                        else:
                            nc.scalar.copy(po[:], pp[:])
                        nc.sync.dma_start(
                            out=out[dt_ * P:(dt_ + 1) * P,
                                    j * QB:(j + 1) * QB],
                            in_=po[:])

                pending_proj = None
                for j in range(NJ):
                    nkt = 4 * j + 4
                    o_ps = [ops_.tile([P, QB], F32, tag="ops",
                                      name=f"ops_{j}_{h}")
                            for h in range(HPC)]
                    sum_ps = [sums.tile([1, QB], F32, tag="sums",
                                        name=f"sums_{j}_{h}")
                              for h in range(HPC)]

                    exps = {}

                    def s_step(h, kt, j=j):
                        """score matmul + exp + diagonal mask for (h, kt)."""
                        s_ps = sps.tile([P, QB], F32, tag="sps",
                                        name=f"sps_{j}_{h}_{kt}")
                        nc.tensor.matmul(
                            s_ps[:],
                            kT_[:, h, kt * P:(kt + 1) * P],
                            qT[:, h, j * QB:(j + 1) * QB],
                            start=True, stop=True)
                        e = p2exp.tile([P, QB], F32R, tag="exp",
                                       name=f"exp_{j}_{h}_{kt}")
                        nc.scalar.activation(
                            e[:], s_ps[:], mybir.ActivationFunctionType.Exp)
                        if kt >= 4 * j:   # diagonal band: zero masked entries
                            off = 384 - P * (kt - 4 * j)
                            nc.vector.tensor_mul(
                                e[:], e[:], mask_sb[:, off:off + QB])
                        exps[(h, kt)] = e

                    # software pipeline: scores run two kt ahead so the
                    # exp-waiting sum/AV matmuls never block the PE queue
                    for h in range(HPC):
                        s_step(h, 0)
                    if nkt > 1:
                        for h in range(HPC):
                            s_step(h, 1)
                    for kt in range(nkt):
                        # previous j's c_proj goes behind our prologue and
                        # first full step so its o_sb dependency (the
                        # softmax-normalize chain) never blocks the PE queue
                        if kt == 1 and pending_proj is not None:
                            emit_proj(*pending_proj)
                            pending_proj = None
                        for h in range(HPC):
                            if kt + 2 < nkt:
                                s_step(h, kt + 2)
                            e = exps.pop((h, kt))
                            nc.tensor.matmul(sum_ps[h][:],
                                             ones_sb[:], e[:],
                                             start=(kt == 0),
                                             stop=(kt == nkt - 1))
                            nc.tensor.matmul(o_ps[h][:], v_sb[:, h, kt, :],
                                             e[:],
                                             start=(kt == 0),
                                             stop=(kt == nkt - 1))
                    o_sb_pair = []
                    for h in range(HPC):
                        rsum = p2sc.tile([1, QB], F32, tag="rsum")
                        rscr = p2sc.tile([1, QB], F32, tag="rscr")
                        nc.vector.reciprocal_approx_accurate(
                            rsum[:], sum_ps[h][:], rscr[:])
                        bc = p2sb.tile([P, QB], F32, tag="bc")
                        nc.gpsimd.partition_broadcast(bc[:], rsum[:])
                        o_sb = p2sb.tile([P, QB], F32R, tag="osb")
                        nc.vector.tensor_mul(o_sb[:], o_ps[h][:], bc[:])
                        o_sb_pair.append(o_sb)

                    if j == NJ - 1:
                        emit_proj(j, o_sb_pair)
                    else:
                        pending_proj = (j, o_sb_pair)

    nc.compile()
    return nc


_NC = None


def _get_nc():
    global _NC
    if _NC is None:
        _NC = _build()
    return _NC


def _host_inputs(x, vi, Wq, Wk, Wv, Wproj, lamb):
    """Build the per-core input maps (all numpy float32)."""
    x = np.asarray(x, dtype=np.float32).reshape(T, D)
    vi = np.asarray(vi, dtype=np.float32).reshape(T, NH, HD)
    Wq = np.asarray(Wq, dtype=np.float32)
    Wk = np.asarray(Wk, dtype=np.float32)
    Wv = np.asarray(Wv, dtype=np.float32)
    Wproj = np.asarray(Wproj, dtype=np.float32)
    lam = float(np.asarray(lamb))

    # x^T tiled: xt[tt, p, kt, f] = x[tt*P+f, kt*P+p]
    xt = np.ascontiguousarray(
        x.reshape(NT, P, KT, P).transpose(0, 3, 2, 1))

    # rope tables
    inv_freq = (1.0 / 10000.0) ** (np.arange(0, HD, 2, dtype=np.float32) / HD)
    tpos = np.arange(T, dtype=np.float32)
    freqs = np.outer(tpos, inv_freq).astype(np.float32)      # [T, 64]
    cs_full = np.concatenate([np.cos(freqs), np.sin(freqs)], axis=1)  # [T,128]
    cs_t = np.ascontiguousarray(
        cs_full.reshape(NT, P, HD).transpose(1, 0, 2))       # [P, NT, HD]

    # causal 0/1 mask bands: column c of slice offset (384-r) maps to
    # f-r = c-384; entry masked iff p > c-384
    m01 = (np.arange(P)[:, None] <= (np.arange(896)[None, :] - 384)
           ).astype(np.float32)
    ones = np.ones((P, 1), dtype=np.float32)

    in_maps = []
    for core in range(NCORES):
        r0 = core * HPC * HD
        wq_c = Wq[r0:r0 + HPC * HD]
        wk_c = Wk[r0:r0 + HPC * HD]
        wv_c = Wv[r0:r0 + HPC * HD] * (1.0 - lam)
        w_all = np.concatenate(
            [wq_c[0:HD], wq_c[HD:2 * HD],
             wk_c[0:HD], wk_c[HD:2 * HD],
             wv_c[0:HD], wv_c[HD:2 * HD]], axis=0)           # [768, D]
        # wall[p, kt, m] = w_all[m, kt*P+p]  (W_all^T tiled)
        wall_c = np.ascontiguousarray(
            w_all.reshape(6 * HD, KT, P).transpose(2, 1, 0))
        # wproj[p, ct, m] = Wproj[m, r0 + ct*P + p]
        wp = Wproj[:, r0:r0 + HPC * HD]                       # [D, 256]
        wproj_c = np.ascontiguousarray(
            wp.reshape(D, HPC, P).transpose(2, 1, 0))
        # vilam[tt, p, c] = lam * vi[tt*P+p, head, hd]
        vl = (lam * vi[:, HPC * core:HPC * (core + 1), :]).reshape(
            NT, P, HPC * HD)
        in_maps.append({
            "xt": xt,
            "wall": wall_c,
            "wproj": wproj_c,
            "vilam": np.ascontiguousarray(vl),
            "cs": cs_t,
            "mask01": m01,
            "ones": ones,
            "identr": np.eye(P, dtype=np.float32),
        })
    return in_maps


def kernel(x, vi, Wq, Wk, Wv, Wproj, lamb, _trace=False, _trace_kwargs=None):
    nc = _get_nc()
    in_maps = _host_inputs(x, vi, Wq, Wk, Wv, Wproj, lamb)
    res = run_bass_kernel_spmd(nc, in_maps, list(range(NCORES)),
                               trace=_trace, **(_trace_kwargs or {}))
    acc = np.zeros((D, T), dtype=np.float64)
    for core in range(NCORES):
        acc += res.results[core]["out"].astype(np.float64)
    y = acc.T.astype(np.float32).reshape(1, 1, T, D)
    if _trace:
        return y, res
    return y

